# revision 7
# baseline (speedup 1.0000x reference)
"""Trainium2 Bass kernel for nn_AttentionBlock (N=32, T=1024, C=K=V=512).

Data-parallel over batch N across 8 NeuronCores (4 batches/core), no
collectives. Per core, per batch:

  xT = transpose(x) via PE matmuls against an fp8 identity (plain matmul,
       f32 PSUM; 4 tiles chained per PSUM bank to respect the 2KB
       zero-region granularity), drains split Act/DVE
  qT/kT/v: fp8e4 DoubleRow matmuls (256-deep contraction per instruction,
       2x bf16 PE throughput); q bias fused into a DVE tensor_scalar drain,
       k bias into an Act activation-Identity drain, v bias into a DVE
       tensor_tensor drain
  scoresT[s,t] = k q^T on causal segments only (DoubleRow); strict lower
       triangle of the diagonal block masked with -1e9 (DVE add)
  attnT = exp(scoresT/sqrt(K)) in one Act pass per key chunk with the row
       sum via accum_out (softmax over the query axis t, per reference)
  vs = v * (64/rowsum) on Pool (single tensor_scalar, two scalar ops);
       the 64x pre-scale keeps vs out of the fp8 subnormal range and is
       divided back out in the attn-out drain
  attn_out columns via DoubleRow pairs over key chunks (odd tails as plain
       fp8 matmuls); PSUM drained on Act with the 1/64 fold
  out = [x (exact f32 echo), attn_out]

Software pipelining across batches: x loads run 3 batches ahead, fp8 casts
and PE transposes 2 ahead, and q/k/v projections 1 ahead (emitted between
scores and attn@v of the previous batch) so the vs8->attnv->drain latency
chain hides under the next batch's projection work. PSUM pools are
segregated by phase (proj/scores-big vs scores-small/transpose vs
v/attn-out) to avoid cross-phase rotation coupling.
"""

import contextlib
import math

import numpy as np

import concourse.bass as bass
import concourse.tile as tile
from bass_rust import add_dep_helper
from concourse import bacc, mybir
from concourse.bass_utils import run_bass_kernel_spmd

N, T, C, K, V = 32, 1024, 512, 512, 512
NCORES = 8
NB = N // NCORES  # batches per core
P = 128
CO = C // P  # 4 chunks of contraction dim
KO = K // P  # 4 chunks of qk feature dim
TO = T // P  # 8 chunks of sequence dim
F32 = mybir.dt.float32
BF16 = mybir.dt.bfloat16
FP8 = mybir.dt.float8e4
SCALE = 1.0 / math.sqrt(K)
NEG = -1.0e9
OSCALE = 64.0  # vs pre-scale to avoid fp8 subnormals; folded out at o drain
DR = mybir.MatmulPerfMode.DoubleRow
ABLATE = set()  # {"attnv", "scores", "proj", "tr"} for HW phase ablation
INLINE_LAST = False


def _body(nc, tc, x_ext, w_exts, b_exts, out_ext, reps=1):
    ctxs = []

    def pool(name, bufs, space="SBUF"):
        p = tc.tile_pool(name=name, bufs=bufs, space=space)
        ctxs.append(p)
        return p.__enter__()

    consts = pool("consts", 1)
    wstage = pool("wstage", 2)
    xn_pool = pool("xn", 4)
    x8_pool = pool("x8", 2)
    xt8_pool = pool("xt8", 3)
    qk_pool = pool("qk", 2)
    at_pool = pool("at", 2)
    small = pool("small", 4)
    ob_pool = pool("ob", 2)
    ppb = pool("ppb", 2, space="PSUM")  # 2-bank [128,1024]: proj/scores-big/tr
    pss_pool = pool("pss", 2, space="PSUM")  # 1-bank [128,512]: scores i>=4
    pav = pool("pav", 2, space="PSUM")  # 1-bank [128,512]: v + attnv
    pools = (
        xn_pool, x8_pool, xt8_pool, qk_pool, at_pool, small,
        ob_pool, ppb, pss_pool, pav,
    )

    # ---- constants ----
    # maskbias[s_local, t_local]: 0 where t >= s, NEG where t < s
    maskbias = consts.tile([P, P], F32)
    nc.gpsimd.memset(maskbias, 0.0)
    nc.gpsimd.affine_select(
        out=maskbias,
        in_=maskbias,
        compare_op=mybir.AluOpType.is_ge,
        fill=NEG,
        base=0,
        pattern=[[1, P]],  # +1 per t (free)
        channel_multiplier=-1,  # -1 per s (partition); keep where t - s >= 0
    )
    warm_lhs = consts.tile([P, P], BF16, tag="warm_lhs")
    nc.gpsimd.memset(warm_lhs, 0.0)
    # fp8 identity for PE-side transpose (plain matmul, lhsT.T @ I).
    # affine_select fills are broken for 8/16-bit dtypes on HW (the fill
    # lands as a wrong constant), so build in f32 and cast.
    ident_f = consts.tile([P, P], F32, tag="ident_f")
    nc.gpsimd.memset(ident_f, 1.0)
    nc.gpsimd.affine_select(
        out=ident_f,
        in_=ident_f,
        compare_op=mybir.AluOpType.is_ge,
        fill=0.0,
        base=0,
        pattern=[[1, P]],
        channel_multiplier=-1,
    )
    nc.gpsimd.affine_select(
        out=ident_f,
        in_=ident_f,
        compare_op=mybir.AluOpType.is_ge,
        fill=0.0,
        base=0,
        pattern=[[-1, P]],
        channel_multiplier=1,
    )
    ident8 = consts.tile([P, P], FP8, tag="ident8")
    nc.vector.tensor_copy(out=ident8, in_=ident_f)

    def load_w(name, w_ext, defer_anchor=None):
        stage = wstage.tile([P, CO, 512], F32, tag="wstage", name=f"stage_{name}")
        dma = nc.sync.dma_start(
            out=stage, in_=w_ext.rearrange("(co p) k -> p co k", p=P)
        )
        if defer_anchor is not None:
            add_dep_helper(dma.ins, defer_anchor.ins, reason="defer behind xT chain")
        w8 = consts.tile([P, CO, 512], FP8, tag=f"w_{name}", name=f"w_{name}")
        nc.vector.tensor_copy(out=w8, in_=stage)
        return w8

    w_8s = [None, None, None]
    bq_t = consts.tile([P, KO], F32, tag="bq")
    bk_t = consts.tile([P, KO], F32, tag="bk")
    bv_b = consts.tile([P, V], F32, tag="bv")

    def early_setup():
        w_8s[0] = load_w("q", w_exts[0])
        nc.sync.dma_start(out=bq_t, in_=b_exts[0].rearrange("(ko p) -> p ko", p=P))

    def late_setup(anchor):
        w_8s[1] = load_w("k", w_exts[1], anchor)
        w_8s[2] = load_w("v", w_exts[2], anchor)
        dma = nc.sync.dma_start(
            out=bk_t, in_=b_exts[1].rearrange("(ko p) -> p ko", p=P)
        )
        if anchor is not None:
            add_dep_helper(dma.ins, anchor.ins, reason="defer behind xT chain")
        bv_src = bass.AP(
            tensor=b_exts[2].tensor,
            offset=b_exts[2].offset,
            ap=[[0, P]] + list(b_exts[2].ap),
        )
        dma = nc.sync.dma_start(out=bv_b, in_=bv_src)
        if anchor is not None:
            add_dep_helper(dma.ins, anchor.ins, reason="defer behind xT chain")

    # PE p-state warmup: >3us of continuous matmul while the first x chain
    # runs; outside the reps loop so steady-state reps don't pay for it.
    scratch = consts.tile([P, 512], BF16, tag="warm_rhs", name="warm_rhs")
    nc.vector.memset(scratch, 0.0)
    wpsum = ppb.tile([P, 1024], F32, tag="psb", name="warm_ps")
    nwarm = 10
    for d in range(nwarm):
        nc.tensor.matmul(
            wpsum[:, 0:512], lhsT=warm_lhs, rhs=scratch,
            start=(d == 0), stop=(d == nwarm - 1),
        )

    loop = tc.For_i(0, reps, 1) if reps > 1 else contextlib.nullcontext()
    with loop:
        _batches(
            nc, tc, x_ext, out_ext, w_8s, bq_t, bk_t, bv_b, maskbias, warm_lhs,
            ident8, pools, early_setup, late_setup,
        )

    for p in reversed(ctxs):
        p.__exit__(None, None, None)


def _batches(
    nc, tc, x_ext, out_ext, w_8s, bq_t, bk_t, bv_b, maskbias, warm_lhs,
    ident8, pools, early_setup, late_setup,
):
    (
        xn_pool, x8_pool, xt8_pool, qk_pool, at_pool, small,
        ob_pool, ppb, pss_pool, pav,
    ) = pools

    def load_stage(n, chunks=2):
        """x load (f32) on SP HWDGE; keep f32 for the exact echo."""
        x_nat = xn_pool.tile([P, TO, C], F32, tag="x_nat", name=f"x_nat_{n}")
        x_view = x_ext[n].rearrange("(to p) c -> p to c", p=P)
        step = TO // chunks
        for h in range(chunks):
            sl = slice(h * step, (h + 1) * step)
            nc.sync.dma_start(out=x_nat[:, sl, :], in_=x_view[:, sl, :])
        return x_nat

    def cast_stage(n, x_nat, chunks=2):
        """Pool f32 -> fp8 cast, emitted where Pool has slack."""
        x8 = x8_pool.tile([P, TO, C], FP8, tag="x8", name=f"x8_{n}")
        step = TO // chunks
        for h in range(chunks):
            sl = slice(h * step, (h + 1) * step)
            nc.gpsimd.tensor_copy(out=x8[:, sl, :], in_=x_nat[:, sl, :])
        return x8

    def tr_stage(n, x8, alternate=False):
        """PE transpose (plain matmul vs fp8 identity, 4 tiles chained per
        PSUM bank) -> DVE drain to fp8 xT8 (alternate Act/DVE at startup)."""
        xT8 = xt8_pool.tile([P, CO, T], FP8, tag="xT8", name=f"xT8_{n}")
        last = None
        # bank-major: all bank-0 transposes only need the first half of the
        # x8 cast, so the PE starts as soon as half the cast lands
        for bank in range(2 if "tr" not in ABLATE else 0):
            for co in range(CO):
                pt = pss_pool.tile(
                    [P, 512], F32, tag="pss", name=f"ptr_{n}_{co}_{bank}"
                )
                for tj in range(4):
                    to = 4 * bank + tj
                    nc.tensor.matmul(
                        pt[:, P * tj : P * (tj + 1)],
                        lhsT=x8[:, to, P * co : P * (co + 1)],
                        rhs=ident8,
                        start=(tj == 0),
                        stop=(tj == 3),
                        skip_group_check=True,
                    )
                dst = xT8[:, co, 512 * bank : 512 * (bank + 1)]
                if (2 * co + bank) % 2 == 0:
                    last = nc.scalar.copy(out=dst, in_=pt)
                else:
                    last = nc.vector.tensor_copy(out=dst, in_=pt)
        return xT8, last

    def attnv_col(n, j, attnT8, vs8, o_f32, o_view):
        ps = pav.tile([P, 512], F32, tag="pav", name=f"psav_{n}_{j}")
        npair = (j + 1) // 2
        odd = (j + 1) % 2 == 1
        for ip in range(npair):
            nc.tensor.matmul(
                ps,
                lhsT=attnT8[:, 2 * ip : 2 * ip + 2, P * j : P * (j + 1)],
                rhs=vs8[:, 2 * ip : 2 * ip + 2, :],
                start=(ip == 0),
                stop=(ip == npair - 1) and not odd,
                perf_mode=DR,
            )
        if odd:
            nc.tensor.matmul(
                ps,
                lhsT=attnT8[:, j, P * j : P * (j + 1)],
                rhs=vs8[:, j, :],
                start=(npair == 0),
                stop=True,
            )
        nc.scalar.mul(out=o_f32[:, j, :], in_=ps, mul=1.0 / OSCALE)
        if j % 2 == 1:
            g = slice(j - 1, j + 1)
            nc.scalar.dma_start(out=o_view[:, g, :], in_=o_f32[:, g, :])

    def proj_v(n, xT8):
        """q/k/v projections for batch n (DoubleRow fp8, bias in drains)."""
        qT8 = qk_pool.tile([P, KO, T], FP8, tag="qT8", name=f"qT8_{n}")
        kT8 = qk_pool.tile([P, KO, T], FP8, tag="kT8", name=f"kT8_{n}")
        for w8, b_t, dst, wname in () if "proj" in ABLATE else (
            (w_8s[0], bq_t, qT8, "q"),
            (w_8s[1], bk_t, kT8, "k"),
        ):
            for ko in range(KO):
                ps = ppb.tile([P, 1024], F32, tag="psb", name=f"psp_{n}_{wname}_{ko}")
                for th in range(2):
                    for cp in range(0, CO, 2):
                        nc.tensor.matmul(
                            ps[:, 512 * th : 512 * (th + 1)],
                            lhsT=w8[:, cp : cp + 2, P * ko : P * (ko + 1)],
                            rhs=xT8[:, cp : cp + 2, 512 * th : 512 * (th + 1)],
                            start=(cp == 0),
                            stop=(cp == CO - 2),
                            perf_mode=DR,
                        )
                if wname == "q":
                    # DVE drain: decouples from Act so next-batch q-proj can
                    # overlap this batch's exp phase
                    nc.vector.tensor_scalar_add(
                        out=dst[:, ko, :], in0=ps, scalar1=b_t[:, ko : ko + 1]
                    )
                else:
                    nc.scalar.activation(
                        out=dst[:, ko, :],
                        in_=ps,
                        func=mybir.ActivationFunctionType.Identity,
                        bias=b_t[:, ko : ko + 1],
                    )
        v_bf = qk_pool.tile([P, TO, V], BF16, tag="v", name=f"v_{n}")
        for so in range(TO if "proj" not in ABLATE else 0):
            ps = pss_pool.tile([P, 512], F32, tag="pss", name=f"psv_{n}_{so}")
            for cp in range(0, CO, 2):
                nc.tensor.matmul(
                    ps,
                    lhsT=xT8[:, cp : cp + 2, P * so : P * (so + 1)],
                    rhs=w_8s[2][:, cp : cp + 2, :],
                    start=(cp == 0),
                    stop=(cp == CO - 2),
                    perf_mode=DR,
                )
            nc.vector.tensor_tensor(
                out=v_bf[:, so, :], in0=ps, in1=bv_b, op=mybir.AluOpType.add
            )
        return qT8, kT8, v_bf

    # Software pipeline: loads run 3 batches ahead, fp8 casts 2 ahead, and
    # transposes 2 ahead (emitted after scores), so xT8(n) is ready when
    # batch n starts and the batch cadence is engine-bound, not chain-bound.
    x_nats = {0: load_stage(0, chunks=4)}
    if early_setup is not None:
        early_setup()
    x8s = {0: cast_stage(0, x_nats[0], chunks=4)}
    xT8_0, last0 = tr_stage(0, x8s[0], alternate=True)
    xT8s = {0: xT8_0}
    if late_setup is not None:
        late_setup(last0)
    for m in (1, 2):
        if m < NB:
            x_nats[m] = load_stage(m)
    x8s[1] = cast_stage(1, x_nats[1])
    xT8_1, _ = tr_stage(1, x8s[1], alternate=True)
    xT8s[1] = xT8_1
    pv = {0: proj_v(0, xT8s[0])}
    for n in range(NB):
        x_nat = x_nats[n]
        qT8, kT8, v_bf = pv[n]
        if n + 3 < NB:
            x_nats[n + 3] = load_stage(n + 3)
        if n + 2 < NB:
            x8s[n + 2] = cast_stage(n + 2, x_nats[n + 2])

        # ---- scores + masked softmax over t (free axis) ----
        attnT8 = at_pool.tile([P, TO, T], FP8, tag="attnT8", name=f"attnT8_{n}")
        vs8 = qk_pool.tile([P, TO, V], FP8, tag="vs8", name=f"vs8_{n}")
        recips = small.tile([P, TO], F32, tag="recips", name=f"recips_{n}")
        o_view = out_ext[n, :, C : C + V].rearrange("(to p) c -> p to c", p=P)
        o_f32 = ob_pool.tile([P, TO, V], F32, tag="o", name=f"o_{n}")
        first_exp = None
        for i in range(TO if "scores" not in ABLATE else 0):
            lo = P * i
            if i < 4:
                ps_t = ppb.tile([P, 1024], F32, tag="psb", name=f"pss_{n}_{i}")
                base = 0
                segs = [(lo, 512), (512, 1024)]
            else:
                ps_t = pss_pool.tile([P, 512], F32, tag="pss", name=f"pss_{n}_{i}")
                base = 512
                segs = [(lo, 1024)]
            for a, b in segs:
                for kp in range(0, KO, 2):
                    nc.tensor.matmul(
                        ps_t[:, a - base : b - base],
                        lhsT=kT8[:, kp : kp + 2, lo : lo + P],
                        rhs=qT8[:, kp : kp + 2, a:b],
                        start=(kp == 0),
                        stop=(kp == KO - 2),
                        perf_mode=DR,
                    )
            nc.vector.tensor_tensor(
                out=ps_t[:, lo - base : lo - base + P],
                in0=ps_t[:, lo - base : lo - base + P],
                in1=maskbias,
                op=mybir.AluOpType.add,
            )
            acc = small.tile([P, 1], F32, tag="acc", name=f"acc_{n}_{i}")
            exp_inst = nc.scalar.activation(
                out=attnT8[:, i, lo:T],
                in_=ps_t[:, lo - base : 1024 - base],
                func=mybir.ActivationFunctionType.Exp,
                scale=SCALE,
                accum_out=acc,
            )
            if first_exp is None:
                first_exp = exp_inst
            nc.vector.reciprocal(out=recips[:, i : i + 1], in_=acc)
            nc.gpsimd.tensor_scalar(
                out=vs8[:, i, :],
                in0=v_bf[:, i, :],
                scalar1=recips[:, i : i + 1],
                scalar2=OSCALE,
                op0=mybir.AluOpType.mult,
                op1=mybir.AluOpType.mult,
            )
            if INLINE_LAST and n == NB - 1 and "attnv" not in ABLATE:
                attnv_col(n, i, attnT8, vs8, o_f32, o_view)

        if n + 1 < NB:
            pv[n + 1] = proj_v(n + 1, xT8s[n + 1])
        if n + 2 < NB:
            xT8s[n + 2], _ = tr_stage(n + 2, x8s[n + 2])

        xcopy = nc.sync.dma_start(
            out=out_ext[n, :, 0:C].rearrange("(to p) c -> p to c", p=P), in_=x_nat
        )
        if n < NB - 1 and first_exp is not None:
            add_dep_helper(
                xcopy.ins, first_exp.ins, reason="defer x-copy behind scores"
            )

        # ---- attn @ v (last batch: inlined in the scores loop) ----
        if n < NB - 1 or not INLINE_LAST:
            for j in range(TO if "attnv" not in ABLATE else 0):
                attnv_col(n, j, attnT8, vs8, o_f32, o_view)

def build_nc(reps=1):
    nc = bacc.Bacc("TRN2", target_bir_lowering=False, debug=False, num_devices=NCORES)
    x_ext = nc.dram_tensor("x", [NB, T, C], F32, kind="ExternalInput").ap()
    wq = nc.dram_tensor("Wq", [C, K], F32, kind="ExternalInput").ap()
    bq = nc.dram_tensor("bq", [K], F32, kind="ExternalInput").ap()
    wk = nc.dram_tensor("Wk", [C, K], F32, kind="ExternalInput").ap()
    bk = nc.dram_tensor("bk", [K], F32, kind="ExternalInput").ap()
    wv = nc.dram_tensor("Wv", [C, V], F32, kind="ExternalInput").ap()
    bv = nc.dram_tensor("bv", [V], F32, kind="ExternalInput").ap()
    out_ext = nc.dram_tensor("out", [NB, T, C + V], F32, kind="ExternalOutput").ap()

    with tile.TileContext(nc) as tc:
        _body(nc, tc, x_ext, (wq, wk, wv), (bq, bk, bv), out_ext, reps=reps)
    nc.compile()
    return nc


def make_in_maps(x, Wq, bq, Wk, bk, Wv, bv):
    x = np.ascontiguousarray(np.asarray(x, dtype=np.float32))
    return [
        {
            "x": x[NB * i : NB * (i + 1)],
            "Wq": np.asarray(Wq, np.float32),
            "bq": np.asarray(bq, np.float32),
            "Wk": np.asarray(Wk, np.float32),
            "bk": np.asarray(bk, np.float32),
            "Wv": np.asarray(Wv, np.float32),
            "bv": np.asarray(bv, np.float32),
        }
        for i in range(NCORES)
    ]


def kernel(x, Wq, bq, Wk, bk, Wv, bv):
    nc = build_nc()
    in_maps = make_in_maps(x, Wq, bq, Wk, bk, Wv, bv)
    res = run_bass_kernel_spmd(nc, in_maps, list(range(NCORES)))
    return np.concatenate([res.results[i]["out"] for i in range(NCORES)], axis=0)
